# revision 23
# baseline (speedup 1.0000x reference)
"""LoRA layer kernel for Trainium2 (8 NeuronCores, data-parallel over rows).

Computes out = ((x @ V^T) * S) @ U^T  (scaling = alpha/rank = 1.0)
for x [4, 2048, 4096], U [4096, 32], S [32], V [32, 4096], all fp32.

Sharding: batch*seq rows (8192) split evenly across the 8 cores; the tiny
LoRA factors are replicated (host pre-transposes them — cheap layout prep).

Per core (1024 rows), fully fp32-exact (rel err ~3e-7):
  - stream x in 256-row chunks (2 row-tiles of 128 = PE partition dim)
  - PE-transpose each 128x128 block of x (fp32 transpose is exact) to get
    features onto partitions; PSUM->SBUF copies alternate DVE/ScalarE
  - mm1: hT[32, 256] += VsT[:, ft, :]^T @ xT(ft), accumulated in one PSUM
    bank over the 32 feature tiles (single N=256 matmul per tile)
  - S is applied during the PSUM->SBUF copy of hT (broadcast multiply)
  - mm2: out[128, 512] = hT-slice^T @ UsT-slice, 16 matmuls per chunk,
    software-pipelined one chunk behind mm1 so they fill PE gaps left by
    transpose-copy waits; out copies alternate ScalarE/DVE
  - per-row-tile DMA stores; the final chunk stores in column halves so the
    last DMAs overlap the mm2 tail
The kernel is PE-bound (this part runs the PE at ~1.2 GHz fixed; fp32
matmuls are 2-pass), so DMA (~94 us roofline) hides entirely under compute.
No collectives needed.
"""

import sys

for _p in ("/root/.axon_site/_ro/trn_rl_repo", "/opt/trn_rl_repo"):
    if _p not in sys.path:
        sys.path.append(_p)

import numpy as np

import concourse.bass as bass
from concourse import mybir
from concourse.bass_utils import run_bass_kernel_spmd
from concourse.tile import TileContext

F32 = mybir.dt.float32
P = 128
ROWS = 1024  # per-core row shard
FEAT = 4096
RANK = 32
CHUNK_TILES = 2
CHUNK = CHUNK_TILES * P  # 256
N_CHUNKS = ROWS // CHUNK  # 4
FT = FEAT // P  # 32
OC = FEAT // 512  # 8
N_CORES = 8


def _split_multiwaits(nc) -> None:
    # Workaround for this container's walrus: engine instructions with >=2
    # sem waits fail codegen ("Too many sync wait commands"). Hoist all but
    # the last wait onto single-wait NoOps inserted just before, same engine.
    for f in nc.m.functions:
        for bb in f.blocks:
            out = []
            changed = False
            for inst in bb.instructions:
                si = inst.sync_info
                waits = list(si.on_wait) if (si is not None and si.on_wait) else []
                if len(waits) > 1:
                    changed = True
                    for w in waits[:-1]:
                        nop = mybir.InstNoOp(name=f"splitw-{nc.next_id()}")
                        nop.engine = inst.engine
                        nop.sync_info = mybir.SyncInfo(on_wait=[w], on_update=[])
                        nc.register_instruction(nop)
                        out.append(nop)
                    si.on_wait = [waits[-1]]
                out.append(inst)
            if changed:
                bb.instructions = out


class _PatchedTileContext(TileContext):
    def _drain_and_barrier(self, tick_clock, wait_clock):
        super()._drain_and_barrier(tick_clock, wait_clock)
        _split_multiwaits(self.nc)


def build_nc() -> bass.Bass:
    nc = bass.Bass(trn_type="TRN2", target_bir_lowering=False, name="lora")
    x_d = nc.dram_tensor("x", [ROWS, FEAT], F32, kind="ExternalInput")
    id_d = nc.dram_tensor("ident", [P, P], F32, kind="ExternalInput")
    # vt pre-tiled on host to [P, FT, RANK] so the DMA is contiguous
    vt_d = nc.dram_tensor("vt", [P, FT * RANK], F32, kind="ExternalInput")
    ut_d = nc.dram_tensor("ut", [RANK, FEAT], F32, kind="ExternalInput")
    s_d = nc.dram_tensor("s", [RANK], F32, kind="ExternalInput")
    out_d = nc.dram_tensor("out", [ROWS, FEAT], F32, kind="ExternalOutput")

    with _PatchedTileContext(nc) as tc:
        with (
            tc.tile_pool(name="consts", bufs=1) as consts,
            tc.tile_pool(name="xin", bufs=2) as x_pool,
            tc.tile_pool(name="xt", bufs=8) as xt_pool,
            tc.tile_pool(name="hts", bufs=2) as h_pool,
            tc.tile_pool(name="outs", bufs=2) as out_pool,
            tc.tile_pool(name="ps_t", bufs=4, space="PSUM") as psum_t,
            tc.tile_pool(name="ps_h", bufs=1, space="PSUM") as psum_h,
            tc.tile_pool(name="ps_o", bufs=3, space="PSUM") as psum_o,
        ):
            # chunk-0 x tiles first so the PE can start ASAP; tile 0 lands
            # in halves so transposes begin after the first 1 MB
            x0_halves = []
            for h in range(2):
                xh = x_pool.tile([P, FEAT // 2], F32, tag=f"x0h{h}")
                nc.sync.dma_start(
                    xh, x_d[0:P, h * (FEAT // 2) : (h + 1) * (FEAT // 2)]
                )
                x0_halves.append(xh)
            x_tiles0 = [None]
            for c in range(1, CHUNK_TILES):
                xt = x_pool.tile([P, FEAT], F32, tag=f"x{c}")
                nc.sync.dma_start(xt, x_d[c * P : (c + 1) * P, :])
                x_tiles0.append(xt)

            ident = consts.tile([P, P], F32)
            nc.sync.dma_start(ident, id_d[:, :])

            vsT = consts.tile([P, FT, RANK], F32)
            nc.sync.dma_start(
                vsT, vt_d[:, :].rearrange("p (kt r) -> p kt r", r=RANK)
            )

            usT = consts.tile([RANK, FEAT], F32)
            s2 = consts.tile([RANK, 1], F32)
            nc.sync.dma_start(usT, ut_d[:, :])
            nc.sync.dma_start(
                s2, s_d[:].rearrange("(r one) -> r one", one=1)
            )

            def emit_mm2_op(hT, out_sb, k):
                oc, c = divmod(k, CHUNK_TILES)
                ps_o = psum_o.tile([P, 512], F32, tag="po")
                nc.tensor.matmul(
                    ps_o,
                    hT[:, c * P : (c + 1) * P],
                    usT[:, oc * 512 : (oc + 1) * 512],
                    start=True,
                    stop=True,
                    skip_group_check=True,
                )
                dst = out_sb[:, c, oc * 512 : (oc + 1) * 512]
                if (oc + c) % 2 == 0:
                    nc.scalar.copy(out=dst, in_=ps_o)
                else:
                    nc.vector.tensor_copy(out=dst, in_=ps_o)

            def store_out(out_sb, ci):
                # per row-tile stores let the DMA start before the whole
                # chunk's copies finish
                for c in range(CHUNK_TILES):
                    r0 = ci * CHUNK + c * P
                    nc.sync.dma_start(
                        out_d[r0 : r0 + P, :],
                        out_sb[:, c, :],
                    )

            N_MM2 = OC * CHUNK_TILES  # 16 mm2 ops per chunk
            pending = None  # (hT, out_sb, ci) of previous chunk
            x_tiles = None

            def x_src(ci, c, ft):
                if ci == 0 and c == 0:
                    half = ft // (FT // 2)
                    off = (ft % (FT // 2)) * P
                    return x0_halves[half][:, off : off + P]
                return x_tiles[c][:, ft * P : (ft + 1) * P]
            for ci in range(N_CHUNKS):
                if ci == 0:
                    x_tiles = x_tiles0
                else:
                    x_tiles = []
                    for c in range(CHUNK_TILES):
                        xt = x_pool.tile([P, FEAT], F32, tag=f"x{c}")
                        r0 = ci * CHUNK + c * P
                        nc.sync.dma_start(xt, x_d[r0 : r0 + P, :])
                        x_tiles.append(xt)

                ps_h = psum_h.tile([RANK, CHUNK], F32, tag="h")
                for ft in range(FT):
                    xTf = xt_pool.tile([P, CHUNK], F32, tag="xTf")
                    ps_t = psum_t.tile([P, CHUNK], F32, tag="pt")
                    for c in range(CHUNK_TILES):
                        nc.tensor.transpose(
                            ps_t[:, c * P : (c + 1) * P],
                            x_src(ci, c, ft),
                            ident,
                        )
                    if ft % 2 == 0:
                        nc.vector.tensor_copy(out=xTf, in_=ps_t)
                    else:
                        nc.scalar.copy(out=xTf, in_=ps_t)
                    nc.tensor.matmul(
                        ps_h,
                        vsT[:, ft, :],
                        xTf,
                        start=(ft == 0),
                        stop=(ft == FT - 1),
                        skip_group_check=True,
                    )
                    # software-pipelined mm2 of the previous chunk (one op
                    # every other ft) fills PE gaps left by copy waits
                    if pending is not None and ft % 2 == 1:
                        emit_mm2_op(pending[0], pending[1], ft // 2)

                if pending is not None:
                    store_out(pending[1], pending[2])

                hT = h_pool.tile([RANK, CHUNK], F32, tag="hT")
                nc.vector.tensor_tensor(
                    hT,
                    ps_h,
                    s2.to_broadcast((RANK, CHUNK)),
                    mybir.AluOpType.mult,
                )
                out_sb = out_pool.tile([P, CHUNK_TILES, FEAT], F32, tag="out")
                pending = (hT, out_sb, ci)

            # tail: last chunk's mm2 runs exclusive; store in halves so the
            # out-DMA overlaps the remaining matmuls
            hT_l, out_l, ci_l = pending
            for k in range(N_MM2):
                emit_mm2_op(hT_l, out_l, k)
                oc, c = divmod(k, CHUNK_TILES)
                if oc == OC // 2 - 1 and c == CHUNK_TILES - 1:
                    for cc in range(CHUNK_TILES):
                        r0 = ci_l * CHUNK + cc * P
                        nc.sync.dma_start(
                            out_d[r0 : r0 + P, : FEAT // 2],
                            out_l[:, cc, : FEAT // 2],
                        )
            for cc in range(CHUNK_TILES):
                r0 = ci_l * CHUNK + cc * P
                nc.sync.dma_start(
                    out_d[r0 : r0 + P, FEAT // 2 :],
                    out_l[:, cc, FEAT // 2 :],
                )
    return nc


_NC_CACHE = None


def _get_nc():
    global _NC_CACHE
    if _NC_CACHE is None:
        _NC_CACHE = build_nc()
    return _NC_CACHE


def make_in_maps(x2, U, S, V):
    vt = np.ascontiguousarray(
        V.T.reshape(FT, P, RANK).transpose(1, 0, 2).reshape(P, FT * RANK)
    )
    ut = np.ascontiguousarray(U.T)
    ident = np.eye(P, dtype=np.float32)
    return [
        {
            "x": np.ascontiguousarray(x2[i * ROWS : (i + 1) * ROWS]),
            "ident": ident,
            "vt": vt,
            "ut": ut,
            "s": np.ascontiguousarray(S, dtype=np.float32),
        }
        for i in range(N_CORES)
    ]


def kernel(**inputs) -> np.ndarray:
    x = np.ascontiguousarray(inputs["x"], dtype=np.float32)
    U = np.ascontiguousarray(inputs["U"], dtype=np.float32)
    S = np.ascontiguousarray(inputs["S"], dtype=np.float32)
    V = np.ascontiguousarray(inputs["V"], dtype=np.float32)

    b, sq, feat = x.shape
    x2 = x.reshape(b * sq, feat)

    nc = _get_nc()
    in_maps = make_in_maps(x2, U, S, V)
    res = run_bass_kernel_spmd(nc, in_maps, core_ids=list(range(N_CORES)))
    out = np.concatenate([r["out"] for r in res.results], axis=0)
    return out.reshape(b, sq, feat)
